# revision 20
# baseline (speedup 1.0000x reference)
"""Trainium2 Bass kernel for nn_GAT_Vanilla (2-layer GAT + BN/ELU + MLP head).

Strategy (8 NeuronCores, graph/data parallel):
- Nodes are sorted by in-degree and striped across 8 cores x 98 blocks x
  128 lanes, so that lane == destination node within a block. A block's
  incoming edges live at (lane, tile) slots with tile count T_g = max
  in-degree of the stripe (degree sorting keeps T_g tight).
- Because lane == dst, the per-block segment-sum is a PSUM accumulation
  of the edge tiles with a CONSTANT identity lhsT on the tensor engine.
- Each edge slot carries 132 bf16 columns: 128 value features (the
  gathered h[src] row) + the 4 per-head src attention scores, gathered
  by the SAME host indexing pass (one table lookup). exp(leaky(s+d)) is
  written into those 4 columns on device and the softmax z-sum rides
  through the same aggregation matmul.
- Per-group work is emitted software-pipelined (head = DMA + score chain
  + alpha multiply runs AHEAD groups early; tail = aggregation +
  normalize + ELU + projections) so the in-order engines always have
  cross-group work queued.
- 3 SPMD launches: A) node projections (x_p, h1, scores) as three
  INDEPENDENT matmuls from x (res_W @ W1 folded on host; h1's constant
  bias row folds into launch B's BN bias); B) layer-1 edge phase +
  layer-2 projection; C) layer-2 edge phase + residual + MLP head +
  log_softmax.
- Between launches the host performs the halo exchange (gather of
  h[src[e]] rows into edge slots - pure indexing/routing, no math) and
  folds BN scales/constants into weight matrices.

Self-contained: only needs numpy + the concourse/bass stack.
"""

import numpy as np

import concourse.bass as bass
import concourse.bacc as bacc
import concourse.tile as tile
from concourse import mybir
from concourse.bass_utils import run_bass_kernel_spmd

F32 = mybir.dt.float32
BF16 = mybir.dt.bfloat16
FP8 = mybir.dt.float8e4

# ---- problem constants (hardcoded per harness contract) ----
N, E, IN, HD, NH, OUT = 100000, 800000, 128, 32, 4, 40
D = HD * NH  # 128
EPS_BN = 1e-5
NEG = -60.0  # pad-edge score -> exp ~ 0

NCORES = 8
NODES_PER_STRIPE = NCORES * 128  # 1024
S = (N + NODES_PER_STRIPE - 1) // NODES_PER_STRIPE  # 98 blocks per core
SLOTS = S * 128  # 12544 node slots per core
DC = D + NH  # 132 packed cols per edge slot (128 value + 4 score/exp)
TCAP = 48   # max (padded) tiles per device group
NBCAP = 12  # max blocks per device group
ACH = 3136  # launch-A nodes per DMA chunk (4 chunks)

PROFILE = False
LAST_EXEC_NS = []

_bf16 = None
_f8 = None


def _bf():
    global _bf16
    if _bf16 is None:
        import ml_dtypes
        _bf16 = ml_dtypes.bfloat16
    return _bf16


def _fp8():
    global _f8
    if _f8 is None:
        import ml_dtypes
        _f8 = ml_dtypes.float8_e4m3fn
    return _f8


# feature permutation: new col f' = c*4 + h  <->  old col f = h*32 + c
PERM = np.array([h * HD + c for c in range(HD) for h in range(NH)],
                dtype=np.int64)


# ----------------------------------------------------------------------------
# Host preprocessing: degree-sorted binning, edge slot layout
# ----------------------------------------------------------------------------

class Prep:
    pass


def host_prep(edge_index):
    """Degree-sorted node striping and per-core edge slot assignment."""
    p = Prep()
    src = np.concatenate([edge_index[0], np.arange(N)]).astype(np.int64)
    dst = np.concatenate([edge_index[1], np.arange(N)]).astype(np.int64)
    deg = np.bincount(dst, minlength=N)  # includes self loop

    order = np.argsort(-deg, kind="stable")  # rank -> node
    rank = np.empty(N, np.int64)
    rank[order] = np.arange(N)
    deg_sorted = deg[order]

    T_list = [int(deg_sorted[s * NODES_PER_STRIPE]) for s in range(S)]

    # group packing: consecutive stripes, uniform padded tile count T_g
    groups = []  # (s0, s1, t0, Tg)
    s0, t0 = 0, 0
    while s0 < S:
        s1, tg = s0 + 1, T_list[s0]
        while (s1 < S and s1 - s0 < NBCAP
               and max(tg, T_list[s1]) * (s1 - s0 + 1) <= TCAP):
            tg = max(tg, T_list[s1])
            s1 += 1
        groups.append((s0, s1, t0, tg))
        t0 += (s1 - s0) * tg
        s0 = s1
    TT = t0
    T_eff = np.zeros(S, np.int64)
    tile_off = np.zeros(S + 1, np.int64)
    for (s0, s1, t0, tg) in groups:
        for i, s in enumerate(range(s0, s1)):
            T_eff[s] = tg
            tile_off[s] = t0 + i * tg
    tile_off[S] = TT
    p.T_list, p.T_eff, p.tile_off, p.TT, p.groups = \
        T_list, T_eff, tile_off, TT, groups
    p.rank, p.order = rank, order

    # edge -> (core, tile, lane) slots
    rv, ru = rank[dst], rank[src]
    eorder = np.argsort(rv, kind="stable")
    rv_s, ru_s = rv[eorder], ru[eorder]
    starts = np.searchsorted(rv_s, np.arange(N))
    j = np.arange(len(rv_s)) - starts[rv_s]
    stripe = rv_s // NODES_PER_STRIPE
    core = (rv_s % NODES_PER_STRIPE) // 128
    lane = rv_s % 128
    etile = tile_off[stripe] + j

    # per-core rank grid: slot (s*128 + l) -> global rank
    base = (np.arange(SLOTS) // 128) * NODES_PER_STRIPE + np.arange(SLOTS) % 128
    p.ranks_c = [base + c * 128 for c in range(NCORES)]  # may exceed N
    p.valid_c = [rc < N for rc in p.ranks_c]

    p.src_idx = []
    for c in range(NCORES):
        m = core == c
        si = np.full((TT, 128), N, np.int32)
        si[etile[m], lane[m]] = ru_s[m]
        p.src_idx.append(si)
    return p


def build_pack(prep, table_u16):
    """Per-core value stream [128, TT, 128] (uint16 view of bf16)."""
    out = []
    for c in range(NCORES):
        v = table_u16[prep.src_idx[c]]  # [TT, 128, 128]
        out.append(np.ascontiguousarray(v.transpose(1, 0, 2)))
    return out


def build_spack(prep, s_rank):
    """Per-core score sidecar [128, TT*4] bf16, (block, head, tile) order
    per group; NEG on pad slots."""
    bf = _bf()
    s_pad = np.concatenate([np.asarray(s_rank, np.float32),
                            np.full((1, NH), NEG, np.float32)]).astype(bf)
    out = []
    for c in range(NCORES):
        sarr = s_pad[prep.src_idx[c]].transpose(1, 0, 2)  # [128, TT, 4]
        P = np.empty((128, prep.TT * NH), bf)
        for (s0, s1, t0, tg) in prep.groups:
            nb = s1 - s0
            nt = nb * tg
            seg = sarr[:, t0:t0 + nt, :].reshape(128, nb, tg, NH)
            seg = np.ascontiguousarray(seg.transpose(0, 1, 3, 2))
            P[:, t0 * NH:(t0 + nt) * NH] = seg.reshape(128, nt * NH)
        out.append(P)
    return out


def build_dtab(prep, d_rank):
    """Per-core dst-score table [128, S, 4] bf16."""
    bf = _bf()
    d_pad = np.concatenate([np.asarray(d_rank, np.float32),
                            np.zeros((1, NH), np.float32)])
    out = []
    for c in range(NCORES):
        arr = d_pad[np.minimum(prep.ranks_c[c], N)].astype(bf)  # [SLOTS, 4]
        out.append(np.ascontiguousarray(
            arr.reshape(S, 128, NH).transpose(1, 0, 2)))
    return out


# ----------------------------------------------------------------------------
# Device kernels
# ----------------------------------------------------------------------------

def build_launch_a():
    """Node projections as 3 independent matmuls from x:
    x_p = x@rw + rb;  h1 = x@rw1 (bias folded downstream);  sd = x@ra + ba.
    """
    nc = bacc.Bacc("TRN2", target_bir_lowering=False, debug=False,
                   num_devices=NCORES)
    xT = nc.dram_tensor("xT", [128, SLOTS], BF16, kind="ExternalInput").ap()
    rw = nc.dram_tensor("rw", [128, 128], BF16, kind="ExternalInput").ap()
    rbcol = nc.dram_tensor("rbcol", [128, 1], F32, kind="ExternalInput").ap()
    rw1 = nc.dram_tensor("rw1", [128, 128], BF16, kind="ExternalInput").ap()
    ra = nc.dram_tensor("ra", [128, 8], BF16, kind="ExternalInput").ap()
    bacol = nc.dram_tensor("bacol", [8, 1], F32, kind="ExternalInput").ap()
    o1 = nc.dram_tensor("o1", [128, 2, SLOTS], BF16,
                        kind="ExternalOutput").ap()
    sdT = nc.dram_tensor("sdT", [8, SLOTS], BF16, kind="ExternalOutput").ap()

    nch = (SLOTS + ACH - 1) // ACH
    AF = mybir.ActivationFunctionType
    ALU = mybir.AluOpType
    with tile.TileContext(nc) as tc:
        with (
            tc.tile_pool(name="const", bufs=1) as cp,
            tc.tile_pool(name="io", bufs=4) as iop,
            tc.tile_pool(name="psa", bufs=2, space="PSUM") as psa,
            tc.tile_pool(name="psb", bufs=2, space="PSUM") as psb,
            tc.tile_pool(name="psc", bufs=2, space="PSUM") as psc,
        ):
            rw_t = cp.tile([128, 128], BF16)
            nc.sync.dma_start(out=rw_t[:], in_=rw)
            rb_t = cp.tile([128, 1], F32)
            nc.sync.dma_start(out=rb_t[:], in_=rbcol)
            rw1_t = cp.tile([128, 128], BF16)
            nc.sync.dma_start(out=rw1_t[:], in_=rw1)
            ra_t = cp.tile([128, 8], BF16)
            nc.sync.dma_start(out=ra_t[:], in_=ra)
            ba_t = cp.tile([8, 1], F32)
            nc.sync.dma_start(out=ba_t[:], in_=bacol)

            for ch in range(nch):
                c0, c1 = ch * ACH, min((ch + 1) * ACH, SLOTS)
                nn = c1 - c0
                xt = iop.tile([128, ACH], BF16, tag="xt")
                nc.sync.dma_start(out=xt[:, 0:nn], in_=xT[:, c0:c1])
                ob = iop.tile([128, 2, ACH], BF16, tag="ob")
                so = iop.tile([8, ACH], BF16, tag="so")
                sub = 0
                for q0 in range(0, nn, 512):
                    q1 = min(q0 + 512, nn)
                    nq = q1 - q0
                    pxp = psa.tile([128, 512], F32, tag="xp")
                    nc.tensor.matmul(out=pxp[:, 0:nq], lhsT=rw_t[:],
                                     rhs=xt[:, q0:q1], start=True, stop=True)
                    ph = psb.tile([128, 512], F32, tag="h")
                    nc.tensor.matmul(out=ph[:, 0:nq], lhsT=rw1_t[:],
                                     rhs=xt[:, q0:q1], start=True, stop=True)
                    psd = psc.tile([8, 512], F32, tag="sd")
                    nc.tensor.matmul(out=psd[:, 0:nq], lhsT=ra_t[:],
                                     rhs=xt[:, q0:q1], start=True, stop=True)
                    nc.scalar.activation(out=ob[:, 0, q0:q1], in_=pxp[:, 0:nq],
                                         func=AF.Identity, bias=rb_t[:])
                    nc.vector.tensor_copy(ob[:, 1, q0:q1], ph[:, 0:nq])
                    if sub % 2 == 0:
                        nc.scalar.activation(out=so[:, q0:q1],
                                             in_=psd[:, 0:nq],
                                             func=AF.Identity, bias=ba_t[:])
                    else:
                        nc.vector.tensor_scalar(out=so[:, q0:q1],
                                                in0=psd[:, 0:nq],
                                                scalar1=ba_t[:], scalar2=None,
                                                op0=ALU.add)
                    sub += 1
                nc.sync.dma_start(out=o1[:, :, c0:c1], in_=ob[:, :, 0:nn])
                nc.gpsimd.dma_start(out=sdT[:, c0:c1], in_=so[:, 0:nn])
    nc.compile()
    return nc


def _edge_inputs(nc, prep):
    aps = {}
    aps["P"] = nc.dram_tensor("P", [128, prep.TT, D], BF16,
                              kind="ExternalInput").ap()
    aps["spack"] = nc.dram_tensor("spack", [128, prep.TT * NH], BF16,
                                  kind="ExternalInput").ap()
    aps["dtab"] = nc.dram_tensor("dtab", [128, S, NH], BF16,
                                 kind="ExternalInput").ap()
    aps["ident"] = nc.dram_tensor("ident", [128, 128], BF16,
                                  kind="ExternalInput").ap()
    aps["crep3"] = nc.dram_tensor("crep3", [128, 3, 128], BF16,
                                  kind="ExternalInput").ap()
    return aps


def _edge_phase(nc, prep, aps, tail_fn, pools):
    """Software-pipelined edge phase (head AHEAD groups early)."""
    cp, iop, wp, up = pools["cp"], pools["iop"], pools["wp"], pools["up"]
    psagg = pools["psagg"]
    ident = cp.tile([128, 128], BF16)
    nc.sync.dma_start(out=ident[:], in_=aps["ident"])
    crep3 = cp.tile([128, 3, 128], BF16)
    nc.sync.dma_start(out=crep3[:], in_=aps["crep3"])
    dtab = cp.tile([128, S, NH], BF16)
    nc.sync.dma_start(out=dtab[:], in_=aps["dtab"])
    spk = cp.tile([128, prep.TT * NH], BF16)
    nc.sync.dma_start(out=spk[:], in_=aps["spack"])
    pools["ident"] = ident

    AF = mybir.ActivationFunctionType
    ALU = mybir.AluOpType
    head_pk = {}
    head_pre = pools.get("head_pre")

    def head(g):
        s0, s1, t0, tg = prep.groups[g]
        nb = s1 - s0
        nt = nb * tg
        pk = iop.tile([128, TCAP, D], BF16, tag="pk")
        nc.sync.dma_start(out=pk[:, 0:nt, :], in_=aps["P"][:, t0:t0 + nt, :])
        if head_pre is not None:
            head_pre(g, s0, nb)

        # alpha chain from the resident score sidecar, (b, h, t) layout
        sc = spk[:, t0 * NH:(t0 + nt) * NH].rearrange(
            "p (b h t) -> p b h t", h=NH, t=tg)
        lg = wp.tile([128, TCAP * NH], BF16, tag="lg")
        lg_v = lg[:, 0:nt * NH].rearrange("p (b h t) -> p b h t", h=NH, t=tg)
        d_b = dtab[:, s0:s1, :].unsqueeze(3).to_broadcast([128, nb, NH, tg])
        nc.gpsimd.tensor_tensor(out=lg_v, in0=sc, in1=d_b, op=ALU.add)
        ll = wp.tile([128, TCAP * NH], BF16, tag="ll")
        nc.scalar.activation(out=ll[:, 0:nt * NH], in_=lg[:, 0:nt * NH],
                             func=AF.Prelu, alpha=0.2)
        ex = wp.tile([128, TCAP * NH], BF16, tag="ex")
        ex_v = ex[:, 0:nt * NH].rearrange("p (b h t) -> p b h t", h=NH, t=tg)
        nc.scalar.activation(out=ex_v,
                             in_=ll[:, 0:nt * NH].rearrange(
                                 "p (b h t) -> p b h t", h=NH, t=tg),
                             func=AF.Exp)
        z = wp.tile([128, NBCAP * NH], F32, tag="z")
        z_v = z[:, 0:nb * NH].rearrange("p (b h) -> p b h", h=NH)
        nc.vector.tensor_reduce(out=z_v, in_=ex_v,
                                axis=mybir.AxisListType.X, op=ALU.add)
        zr = wp.tile([128, NBCAP * NH], BF16, tag="zr")
        zr_v = zr[:, 0:nb * NH].rearrange("p (b h) -> p b h", h=NH)
        with nc.allow_low_precision(reason="softmax z recip; 2e-2 budget"):
            nc.vector.reciprocal(zr_v, z_v)
        # normalized alpha into slot-major layout (strided write on the
        # otherwise-idle Pool engine)
        at = wp.tile([128, TCAP, NH], BF16, tag="at")
        at_bht = at[:, 0:nt, :].rearrange("p (b t) h -> p b h t", t=tg)
        zr_b = zr_v.unsqueeze(3).to_broadcast([128, nb, NH, tg])
        nc.gpsimd.tensor_tensor(out=at_bht, in0=ex_v, in1=zr_b, op=ALU.mult)

        # alpha-weight the value rows ((c,h) order, 2x DVE mode)
        vh = pk[:, 0:nt, :].rearrange("p t (c h) -> p t c h", h=NH)
        ab = at[:, 0:nt, :].unsqueeze(2).to_broadcast([128, nt, HD, NH])
        nc.vector.tensor_tensor(out=vh, in0=vh, in1=ab, op=ALU.mult)
        head_pk[g] = pk

    def tailw(g):
        s0, s1, t0, tg = prep.groups[g]
        nb = s1 - s0
        nt = nb * tg
        pk = head_pk.pop(g)
        vb = pk[:, 0:nt, :].rearrange("p (b t) f -> p b t f", t=tg)
        u_sb = up.tile([128, NBCAP, 128], BF16, tag="u")
        eg = up.tile([128, NBCAP, 128], BF16, tag="eg")
        for q0 in range(0, nb, 3):
            q1 = min(q0 + 3, nb)
            nq = q1 - q0
            ps = psagg.tile([128, 3, 128], F32, tag="agg")
            for t in range(tg):
                nc.tensor.matmul(out=ps[:, 0:nq, :], lhsT=ident[:],
                                 rhs=vb[:, q0:q1, t, :],
                                 start=(t == 0), stop=False)
            nc.tensor.matmul(out=ps[:, 0:nq, :], lhsT=ident[:],
                             rhs=crep3[:, 0:nq, :], start=False, stop=True)
            # ELU pieces straight from PSUM, both on ACT back-to-back
            nc.scalar.activation(out=u_sb[:, q0:q1, :], in_=ps[:, 0:nq, :],
                                 func=AF.Relu)
            nc.scalar.activation(out=eg[:, q0:q1, :], in_=ps[:, 0:nq, :],
                                 func=AF.Exp)
        uf = u_sb[:, 0:nb, :]
        nc.vector.scalar_tensor_tensor(out=uf, in0=eg[:, 0:nb, :],
                                       scalar=-1.0, in1=uf,
                                       op0=ALU.add, op1=ALU.min)
        tail_fn(s0, nb, u_sb, pools)

    G = len(prep.groups)
    AHEAD = 3
    for g in range(min(AHEAD, G)):
        head(g)
    for g in range(G):
        if g + AHEAD < G:
            head(g + AHEAD)
        tailw(g)


def build_launch_b(prep):
    """Layer-1 edge phase + layer-2 projection (h2cat = u1 @ W2cat)."""
    nc = bacc.Bacc("TRN2", target_bir_lowering=False, debug=False,
                   num_devices=NCORES)
    aps = _edge_inputs(nc, prep)
    aps["w2"] = nc.dram_tensor("w2", [128, 136], BF16,
                               kind="ExternalInput").ap()
    h2T = nc.dram_tensor("h2T", [128, SLOTS], BF16,
                         kind="ExternalOutput").ap()
    sd2T = nc.dram_tensor("sd2T", [8, SLOTS], BF16,
                          kind="ExternalOutput").ap()

    with tile.TileContext(nc) as tc:
        with (
            tc.tile_pool(name="const", bufs=1) as cp,
            tc.tile_pool(name="io", bufs=4) as iop,
            tc.tile_pool(name="wk", bufs=3) as wp,
            tc.tile_pool(name="u", bufs=3) as up,
            tc.tile_pool(name="og", bufs=3) as ogp,
            tc.tile_pool(name="psagg", bufs=2, space="PSUM") as psagg,
            tc.tile_pool(name="pst", bufs=2, space="PSUM") as pst,
            tc.tile_pool(name="psh", bufs=2, space="PSUM") as psh,
            tc.tile_pool(name="pssd", bufs=2, space="PSUM") as pssd,
        ):
            pools = dict(cp=cp, iop=iop, wp=wp, up=up, psagg=psagg)
            w2_t = cp.tile([128, 136], BF16)
            nc.sync.dma_start(out=w2_t[:], in_=aps["w2"])

            def tail(s0, nb, u_sb, pools):
                ident = pools["ident"]
                o1g = ogp.tile([128, NBCAP, 128], BF16, tag="o1")
                for i in range(nb):
                    pt = pst.tile([128, 128], BF16, tag="t")
                    nc.tensor.transpose(out=pt[:], in_=u_sb[:, i, :],
                                        identity=ident[:])
                    if i % 2 == 0:
                        nc.scalar.copy(o1g[:, i, :], pt[:])
                    else:
                        nc.vector.tensor_copy(o1g[:, i, :], pt[:])
                h2c = ogp.tile([128, NBCAP, 128], BF16, tag="h2c")
                s2c = ogp.tile([8, NBCAP, 128], BF16, tag="s2c")
                for q0 in range(0, nb, 4):
                    q1 = min(q0 + 4, nb)
                    k = (q1 - q0) * 128
                    rhs = o1g[:, q0:q1, :].rearrange("p b f -> p (b f)")
                    ph = psh.tile([128, 512], F32, tag="h")
                    nc.tensor.matmul(out=ph[:, 0:k], lhsT=w2_t[:, 0:128],
                                     rhs=rhs, start=True, stop=True)
                    nc.scalar.copy(
                        h2c[:, q0:q1, :].rearrange("p b f -> p (b f)"),
                        ph[:, 0:k])
                    psd = pssd.tile([8, 512], F32, tag="sd")
                    nc.tensor.matmul(out=psd[:, 0:k], lhsT=w2_t[:, 128:136],
                                     rhs=rhs, start=True, stop=True)
                    if (q0 // 4) % 2 == 0:
                        nc.scalar.copy(
                            s2c[:, q0:q1, :].rearrange("p b f -> p (b f)"),
                            psd[:, 0:k])
                    else:
                        nc.vector.tensor_copy(
                            s2c[:, q0:q1, :].rearrange("p b f -> p (b f)"),
                            psd[:, 0:k])
                nc.gpsimd.dma_start(out=h2T[:, s0 * 128:(s0 + nb) * 128],
                                    in_=h2c[:, 0:nb, :])
                nc.gpsimd.dma_start(out=sd2T[:, s0 * 128:(s0 + nb) * 128],
                                    in_=s2c[:, 0:nb, :])

            _edge_phase(nc, prep, aps, tail, pools)
    nc.compile()
    return nc


def build_launch_c(prep):
    """Layer-2 edge phase + residual + MLP head + log_softmax."""
    nc = bacc.Bacc("TRN2", target_bir_lowering=False, debug=False,
                   num_devices=NCORES)
    aps = _edge_inputs(nc, prep)
    aps["wc1"] = nc.dram_tensor("wc1", [128, 64], BF16,
                                kind="ExternalInput").ap()
    aps["cccol"] = nc.dram_tensor("cccol", [64, 1], F32,
                                  kind="ExternalInput").ap()
    aps["wc2aug"] = nc.dram_tensor("wc2aug", [65, 40], BF16,
                                   kind="ExternalInput").ap()
    aps["xp"] = nc.dram_tensor("xp", [128, S, 128], BF16,
                               kind="ExternalInput").ap()
    fin = nc.dram_tensor("fin", [128, S, 40], BF16,
                         kind="ExternalOutput").ap()

    AF = mybir.ActivationFunctionType
    ALU = mybir.AluOpType
    with tile.TileContext(nc) as tc:
        with (
            tc.tile_pool(name="const", bufs=1) as cp,
            tc.tile_pool(name="io", bufs=4) as iop,
            tc.tile_pool(name="wk", bufs=3) as wp,
            tc.tile_pool(name="u", bufs=3) as up,
            tc.tile_pool(name="og", bufs=3) as ogp,
            tc.tile_pool(name="psagg", bufs=2, space="PSUM") as psagg,
            tc.tile_pool(name="pst", bufs=2, space="PSUM") as pst,
            tc.tile_pool(name="psr", bufs=2, space="PSUM") as psr,
            tc.tile_pool(name="psy", bufs=2, space="PSUM") as psy,
        ):
            pools = dict(cp=cp, iop=iop, wp=wp, up=up, psagg=psagg)
            wc1_t = cp.tile([128, 64], BF16)
            nc.sync.dma_start(out=wc1_t[:], in_=aps["wc1"])
            cc_t = cp.tile([64, 1], F32)
            nc.sync.dma_start(out=cc_t[:], in_=aps["cccol"])
            wc2_t = cp.tile([65, 40], BF16)
            nc.sync.dma_start(out=wc2_t[:], in_=aps["wc2aug"])

            xpts = {}

            def head_pre(g, s0, nb):
                xpt = iop.tile([128, NBCAP, 128], BF16, tag="xpt")
                xpts[g % 4] = xpt
                nc.sync.dma_start(out=xpt[:, 0:nb, :],
                                  in_=aps["xp"][:, s0:s0 + nb, :])
            pools["head_pre"] = head_pre
            gcount = [0]
            y_all = cp.tile([128, S, 40], BF16)
            zs_all = cp.tile([128, S], F32)

            def tail(s0, nb, u_sb, pools):
                ident = pools["ident"]
                xpt = xpts[gcount[0] % 4]
                gcount[0] += 1
                uf = u_sb[:, 0:nb, :]
                nc.vector.tensor_tensor(out=uf, in0=uf,
                                        in1=xpt[:, 0:nb, :], op=ALU.add)
                o2g = ogp.tile([128, NBCAP, 128], BF16, tag="o2")
                for i in range(nb):
                    pt = pst.tile([128, 128], BF16, tag="t")
                    nc.tensor.transpose(out=pt[:], in_=u_sb[:, i, :],
                                        identity=ident[:])
                    if i % 2 == 0:
                        nc.scalar.copy(o2g[:, i, :], pt[:])
                    else:
                        nc.vector.tensor_copy(o2g[:, i, :], pt[:])
                r1g = ogp.tile([65, NBCAP, 128], BF16, tag="r1")
                nc.gpsimd.memset(r1g[64:65, 0:nb, :], 1.0)
                for q0 in range(0, nb, 4):
                    q1 = min(q0 + 4, nb)
                    k = (q1 - q0) * 128
                    rhs = o2g[:, q0:q1, :].rearrange("p b f -> p (b f)")
                    prt = psr.tile([64, 512], F32, tag="r")
                    nc.tensor.matmul(out=prt[:, 0:k], lhsT=wc1_t[:],
                                     rhs=rhs, start=True, stop=True)
                    nc.scalar.activation(
                        out=r1g[0:64, q0:q1, :].rearrange("p b f -> p (b f)"),
                        in_=prt[:, 0:k], func=AF.Relu, bias=cc_t[:])
                pyg = psy.tile([128, NBCAP, 40], F32, tag="py")
                for i in range(nb):
                    nc.tensor.matmul(out=pyg[:, i, :], lhsT=r1g[:, i, :],
                                     rhs=wc2_t[:], start=True, stop=True)
                # softmax pieces only; Ln deferred to one end-of-launch
                # pass (per-group Ln thrashes the ACT function table)
                egy = wp.tile([128, NBCAP, 40], BF16, tag="egy")
                nc.scalar.activation(out=egy[:, 0:nb, :], in_=pyg[:, 0:nb, :],
                                     func=AF.Exp)
                nc.vector.tensor_reduce(out=zs_all[:, s0:s0 + nb]
                                        .unsqueeze(2), in_=egy[:, 0:nb, :],
                                        axis=mybir.AxisListType.X, op=ALU.add)
                nc.scalar.copy(y_all[:, s0:s0 + nb, :], pyg[:, 0:nb, :])

            _edge_phase(nc, prep, aps, tail, pools)
            lz_all = cp.tile([128, S], F32)
            nc.scalar.activation(out=lz_all[:].unsqueeze(2),
                                 in_=zs_all[:].unsqueeze(2), func=AF.Ln)
            fin_all = cp.tile([128, S, 40], BF16)
            lz_b = lz_all[:].unsqueeze(2).to_broadcast([128, S, 40])
            nc.vector.tensor_tensor(out=fin_all[:], in0=y_all[:], in1=lz_b,
                                    op=ALU.subtract)
            nc.gpsimd.dma_start(out=fin, in_=fin_all[:])
    nc.compile()
    return nc


# ----------------------------------------------------------------------------
# Host orchestration
# ----------------------------------------------------------------------------

_cache = {}


def _get(key, fn):
    if key not in _cache:
        _cache[key] = fn()
    return _cache[key]


def _amat(a):
    """[NH, HD] attention vector -> [128, NH] block matrix."""
    m = np.zeros((D, NH), np.float32)
    for h in range(NH):
        m[h * HD:(h + 1) * HD, h] = a[h]
    return m


def _run(nc, in_maps, tag):
    res = run_bass_kernel_spmd(nc, in_maps, list(range(NCORES)),
                               trace=PROFILE)
    if PROFILE:
        LAST_EXEC_NS.append((tag, res.exec_time_ns))
    return res.results


def _fold_bn(g_, be_, rm_, rv_, bias):
    k = (g_ / np.sqrt(rv_ + EPS_BN)).astype(np.float32)
    c = ((bias - rm_) * k + be_).astype(np.float32)
    return k, c


def kernel(x, edge_index, res_W, res_b,
           W1, as1, ad1, b1, g1, be1, rm1, rv1,
           W2, as2, ad2, b2, g2, be2, rm2, rv2,
           Wc1, bc1, gc, bec, rmc, rvc, Wc2, bc2):
    bf = _bf()
    x = np.asarray(x, np.float32)
    edge_index = np.asarray(edge_index)

    ekey = ("prep", hash(edge_index.tobytes()))
    prep = _get(ekey, lambda: host_prep(edge_index.astype(np.int64)))

    k1, c1 = _fold_bn(g1, be1, rm1, rv1, b1)
    k2, c2 = _fold_bn(g2, be2, rm2, rv2, b2)
    kc, cc = _fold_bn(gc, bec, rmc, rvc, bc1)

    res_W = np.asarray(res_W, np.float32)
    res_b = np.asarray(res_b, np.float32)
    W1k = np.asarray(W1, np.float32) * k1[None, :]
    A1cat = np.concatenate([W1 @ _amat(as1), W1 @ _amat(ad1)], axis=1)
    RW1 = (res_W @ W1k).astype(bf)
    RA = (res_W @ A1cat).astype(bf)
    ba = (res_b @ A1cat).astype(np.float32)
    rb1k = (res_b @ W1k).astype(np.float32)  # h1 bias, folded into crep1
    W2cat = np.concatenate(
        [np.asarray(W2, np.float32) * k2[None, :],
         W2 @ _amat(as2), W2 @ _amat(ad2)], axis=1)[PERM, :].astype(bf)
    Wc1p = (np.asarray(Wc1, np.float32) * kc[None, :])[PERM, :].astype(bf)
    Wc2aug = np.concatenate(
        [np.asarray(Wc2, np.float32),
         np.asarray(bc2, np.float32).reshape(1, OUT)]).astype(bf)
    ident = np.eye(128, dtype=bf)
    rep3 = lambda v: np.tile(np.asarray(v, np.float32).astype(bf),
                             (128, 3)).reshape(128, 3, 128)
    crep3_1 = rep3((c1 + rb1k)[PERM])
    crep3_2 = rep3(c2[PERM])

    # ---- launch A: node projections ----
    x_pad = np.concatenate([x, np.zeros((1, IN), np.float32)]).astype(bf)
    nc_a = _get("A", build_launch_a)
    in_a = []
    for c in range(NCORES):
        idx = np.where(prep.valid_c[c], prep.order[
            np.minimum(prep.ranks_c[c], N - 1)], N)
        xs = np.ascontiguousarray(x_pad[idx].T)
        in_a.append(dict(xT=xs, rw=res_W.astype(bf),
                         rbcol=res_b.reshape(128, 1),
                         rw1=RW1, ra=RA, bacol=ba.reshape(8, 1)))
    res_a = _run(nc_a, in_a, "A")

    # assemble rank-ordered h1 / s1 / d1 / xp
    h1_rank = np.zeros((N, 128), bf)
    sd1_rank = np.zeros((N, 8), np.float32)
    xp_rows = []
    for c in range(NCORES):
        v = prep.valid_c[c]
        rc = prep.ranks_c[c][v]
        o = np.asarray(res_a[c]["o1"])  # [128, 2, SLOTS] bf16
        h1_rank[rc] = o[:, 1, :].T[v]
        sd1_rank[rc] = np.asarray(res_a[c]["sdT"]).T[v].astype(np.float32)
        xr = np.ascontiguousarray(
            o[PERM, 0, :].T.reshape(S, 128, 128).transpose(1, 0, 2))
        xp_rows.append(xr)

    def table_of(h_rank_bf):
        t = np.zeros((N + 1, D), np.uint16)
        t[:N] = h_rank_bf[:, PERM].view(np.uint16)
        return t

    # ---- launch B: layer-1 edge phase + layer-2 projection ----
    P1 = build_pack(prep, table_of(h1_rank))
    s1p = build_spack(prep, sd1_rank[:, 0:4])
    d1 = build_dtab(prep, sd1_rank[:, 4:8])
    nc_b = _get(("B", prep.TT), lambda: build_launch_b(prep))
    in_b = [dict(P=P1[c].view(bf), spack=s1p[c], dtab=d1[c], ident=ident,
                 crep3=crep3_1, w2=W2cat) for c in range(NCORES)]
    res_b_ = _run(nc_b, in_b, "B")

    h2_rank = np.zeros((N, 128), bf)
    sd2_rank = np.zeros((N, 8), np.float32)
    for c in range(NCORES):
        v = prep.valid_c[c]
        rc = prep.ranks_c[c][v]
        h2_rank[rc] = np.asarray(res_b_[c]["h2T"]).T[v]
        sd2_rank[rc] = np.asarray(res_b_[c]["sd2T"]).T[v].astype(np.float32)

    # ---- launch C: layer-2 edge phase + residual + head ----
    P2 = build_pack(prep, table_of(h2_rank))
    s2p = build_spack(prep, sd2_rank[:, 0:4])
    d2 = build_dtab(prep, sd2_rank[:, 4:8])
    nc_c = _get(("C", prep.TT), lambda: build_launch_c(prep))
    in_c = [dict(P=P2[c].view(bf), spack=s2p[c], dtab=d2[c], ident=ident,
                 crep3=crep3_2,
                 wc1=Wc1p, cccol=cc.reshape(64, 1).astype(np.float32),
                 wc2aug=Wc2aug, xp=xp_rows[c])
            for c in range(NCORES)]
    res_c = _run(nc_c, in_c, "C")

    out_rank = np.zeros((N, OUT), np.float32)
    for c in range(NCORES):
        v = prep.valid_c[c]
        rc = prep.ranks_c[c][v]
        f = np.asarray(res_c[c]["fin"]).astype(np.float32)
        out_rank[rc] = f.transpose(1, 0, 2).reshape(SLOTS, OUT)[v]
    out = np.empty((N, OUT), np.float32)
    out[prep.order] = out_rank
    return out


# revision 22
# speedup vs baseline: 1.0174x; 1.0174x over previous
"""Trainium2 Bass kernel for nn_GAT_Vanilla (2-layer GAT + BN/ELU + MLP head).

Strategy (8 NeuronCores, graph/data parallel):
- Nodes are sorted by in-degree and striped across 8 cores x 98 blocks x
  128 lanes, so that lane == destination node within a block. A block's
  incoming edges live at (lane, tile) slots with tile count T_g = max
  in-degree of the stripe (degree sorting keeps T_g tight).
- Because lane == dst, the per-block segment-sum is a PSUM accumulation
  of the edge tiles with a CONSTANT identity lhsT on the tensor engine.
- Each edge slot carries 132 bf16 columns: 128 value features (the
  gathered h[src] row) + the 4 per-head src attention scores, gathered
  by the SAME host indexing pass (one table lookup). exp(leaky(s+d)) is
  written into those 4 columns on device and the softmax z-sum rides
  through the same aggregation matmul.
- Per-group work is emitted software-pipelined (head = DMA + score chain
  + alpha multiply runs AHEAD groups early; tail = aggregation +
  normalize + ELU + projections) so the in-order engines always have
  cross-group work queued.
- 3 SPMD launches: A) node projections (x_p, h1, scores) as three
  INDEPENDENT matmuls from x (res_W @ W1 folded on host; h1's constant
  bias row folds into launch B's BN bias); B) layer-1 edge phase +
  layer-2 projection; C) layer-2 edge phase + residual + MLP head +
  log_softmax.
- Between launches the host performs the halo exchange (gather of
  h[src[e]] rows into edge slots - pure indexing/routing, no math) and
  folds BN scales/constants into weight matrices.

Self-contained: only needs numpy + the concourse/bass stack.
"""

import numpy as np

import concourse.bass as bass
import concourse.bacc as bacc
import concourse.tile as tile
from concourse import mybir
from concourse.bass_utils import run_bass_kernel_spmd

F32 = mybir.dt.float32
BF16 = mybir.dt.bfloat16
FP8 = mybir.dt.float8e4

# ---- problem constants (hardcoded per harness contract) ----
N, E, IN, HD, NH, OUT = 100000, 800000, 128, 32, 4, 40
D = HD * NH  # 128
EPS_BN = 1e-5
NEG = -60.0  # pad-edge score -> exp ~ 0

NCORES = 8
NODES_PER_STRIPE = NCORES * 128  # 1024
S = (N + NODES_PER_STRIPE - 1) // NODES_PER_STRIPE  # 98 blocks per core
SLOTS = S * 128  # 12544 node slots per core
DC = D + NH  # 132 packed cols per edge slot (128 value + 4 score/exp)
TCAP = 64   # max (padded) tiles per device group
NBCAP = 12  # max blocks per device group
ACH = 3136  # launch-A nodes per DMA chunk (4 chunks)

PROFILE = False
LAST_EXEC_NS = []

_bf16 = None
_f8 = None


def _bf():
    global _bf16
    if _bf16 is None:
        import ml_dtypes
        _bf16 = ml_dtypes.bfloat16
    return _bf16


def _fp8():
    global _f8
    if _f8 is None:
        import ml_dtypes
        _f8 = ml_dtypes.float8_e4m3fn
    return _f8


# feature permutation: new col f' = c*4 + h  <->  old col f = h*32 + c
PERM = np.array([h * HD + c for c in range(HD) for h in range(NH)],
                dtype=np.int64)


# ----------------------------------------------------------------------------
# Host preprocessing: degree-sorted binning, edge slot layout
# ----------------------------------------------------------------------------

class Prep:
    pass


def host_prep(edge_index):
    """Degree-sorted node striping and per-core edge slot assignment."""
    p = Prep()
    src = np.concatenate([edge_index[0], np.arange(N)]).astype(np.int64)
    dst = np.concatenate([edge_index[1], np.arange(N)]).astype(np.int64)
    deg = np.bincount(dst, minlength=N)  # includes self loop

    order = np.argsort(-deg, kind="stable")  # rank -> node
    rank = np.empty(N, np.int64)
    rank[order] = np.arange(N)
    deg_sorted = deg[order]

    T_list = [int(deg_sorted[s * NODES_PER_STRIPE]) for s in range(S)]

    # group packing: consecutive stripes, uniform padded tile count T_g
    groups = []  # (s0, s1, t0, Tg)
    s0, t0 = 0, 0
    while s0 < S:
        s1, tg = s0 + 1, T_list[s0]
        while (s1 < S and s1 - s0 < NBCAP
               and max(tg, T_list[s1]) * (s1 - s0 + 1) <= TCAP):
            tg = max(tg, T_list[s1])
            s1 += 1
        groups.append((s0, s1, t0, tg))
        t0 += (s1 - s0) * tg
        s0 = s1
    TT = t0
    T_eff = np.zeros(S, np.int64)
    tile_off = np.zeros(S + 1, np.int64)
    for (s0, s1, t0, tg) in groups:
        for i, s in enumerate(range(s0, s1)):
            T_eff[s] = tg
            tile_off[s] = t0 + i * tg
    tile_off[S] = TT
    p.T_list, p.T_eff, p.tile_off, p.TT, p.groups = \
        T_list, T_eff, tile_off, TT, groups
    p.rank, p.order = rank, order

    # edge -> (core, tile, lane) slots
    rv, ru = rank[dst], rank[src]
    eorder = np.argsort(rv, kind="stable")
    rv_s, ru_s = rv[eorder], ru[eorder]
    starts = np.searchsorted(rv_s, np.arange(N))
    j = np.arange(len(rv_s)) - starts[rv_s]
    stripe = rv_s // NODES_PER_STRIPE
    core = (rv_s % NODES_PER_STRIPE) // 128
    lane = rv_s % 128
    etile = tile_off[stripe] + j

    # per-core rank grid: slot (s*128 + l) -> global rank
    base = (np.arange(SLOTS) // 128) * NODES_PER_STRIPE + np.arange(SLOTS) % 128
    p.ranks_c = [base + c * 128 for c in range(NCORES)]  # may exceed N
    p.valid_c = [rc < N for rc in p.ranks_c]

    p.src_idx = []
    for c in range(NCORES):
        m = core == c
        si = np.full((TT, 128), N, np.int32)
        si[etile[m], lane[m]] = ru_s[m]
        p.src_idx.append(si)
    return p


def build_pack(prep, table_u16):
    """Per-core value stream [128, TT, 128] (uint16 view of bf16)."""
    out = []
    for c in range(NCORES):
        v = table_u16[prep.src_idx[c]]  # [TT, 128, 128]
        out.append(np.ascontiguousarray(v.transpose(1, 0, 2)))
    return out


def build_spack(prep, s_rank):
    """Per-core score sidecar [128, TT*4] bf16, (block, head, tile) order
    per group; NEG on pad slots."""
    bf = _bf()
    s_pad = np.concatenate([np.asarray(s_rank, np.float32),
                            np.full((1, NH), NEG, np.float32)]).astype(bf)
    out = []
    for c in range(NCORES):
        sarr = s_pad[prep.src_idx[c]].transpose(1, 0, 2)  # [128, TT, 4]
        P = np.empty((128, prep.TT * NH), bf)
        for (s0, s1, t0, tg) in prep.groups:
            nb = s1 - s0
            nt = nb * tg
            seg = sarr[:, t0:t0 + nt, :].reshape(128, nb, tg, NH)
            seg = np.ascontiguousarray(seg.transpose(0, 1, 3, 2))
            P[:, t0 * NH:(t0 + nt) * NH] = seg.reshape(128, nt * NH)
        out.append(P)
    return out


def build_dtab(prep, d_rank):
    """Per-core dst-score table [128, S, 4] bf16."""
    bf = _bf()
    d_pad = np.concatenate([np.asarray(d_rank, np.float32),
                            np.zeros((1, NH), np.float32)])
    out = []
    for c in range(NCORES):
        arr = d_pad[np.minimum(prep.ranks_c[c], N)].astype(bf)  # [SLOTS, 4]
        out.append(np.ascontiguousarray(
            arr.reshape(S, 128, NH).transpose(1, 0, 2)))
    return out


# ----------------------------------------------------------------------------
# Device kernels
# ----------------------------------------------------------------------------

def build_launch_a():
    """Node projections as 3 independent matmuls from x:
    x_p = x@rw + rb;  h1 = x@rw1 (bias folded downstream);  sd = x@ra + ba.
    """
    nc = bacc.Bacc("TRN2", target_bir_lowering=False, debug=False,
                   num_devices=NCORES)
    xT = nc.dram_tensor("xT", [128, SLOTS], BF16, kind="ExternalInput").ap()
    rw = nc.dram_tensor("rw", [128, 128], BF16, kind="ExternalInput").ap()
    rbcol = nc.dram_tensor("rbcol", [128, 1], F32, kind="ExternalInput").ap()
    rw1 = nc.dram_tensor("rw1", [128, 128], BF16, kind="ExternalInput").ap()
    ra = nc.dram_tensor("ra", [128, 8], BF16, kind="ExternalInput").ap()
    bacol = nc.dram_tensor("bacol", [8, 1], F32, kind="ExternalInput").ap()
    o1 = nc.dram_tensor("o1", [128, 2, SLOTS], BF16,
                        kind="ExternalOutput").ap()
    sdT = nc.dram_tensor("sdT", [8, SLOTS], BF16, kind="ExternalOutput").ap()

    nch = (SLOTS + ACH - 1) // ACH
    AF = mybir.ActivationFunctionType
    ALU = mybir.AluOpType
    with tile.TileContext(nc) as tc:
        with (
            tc.tile_pool(name="const", bufs=1) as cp,
            tc.tile_pool(name="io", bufs=4) as iop,
            tc.tile_pool(name="psa", bufs=2, space="PSUM") as psa,
            tc.tile_pool(name="psb", bufs=2, space="PSUM") as psb,
            tc.tile_pool(name="psc", bufs=2, space="PSUM") as psc,
        ):
            rw_t = cp.tile([128, 128], BF16)
            nc.sync.dma_start(out=rw_t[:], in_=rw)
            rb_t = cp.tile([128, 1], F32)
            nc.sync.dma_start(out=rb_t[:], in_=rbcol)
            rw1_t = cp.tile([128, 128], BF16)
            nc.sync.dma_start(out=rw1_t[:], in_=rw1)
            ra_t = cp.tile([128, 8], BF16)
            nc.sync.dma_start(out=ra_t[:], in_=ra)
            ba_t = cp.tile([8, 1], F32)
            nc.sync.dma_start(out=ba_t[:], in_=bacol)

            for ch in range(nch):
                c0, c1 = ch * ACH, min((ch + 1) * ACH, SLOTS)
                nn = c1 - c0
                xt = iop.tile([128, ACH], BF16, tag="xt")
                nc.sync.dma_start(out=xt[:, 0:nn], in_=xT[:, c0:c1])
                ob = iop.tile([128, 2, ACH], BF16, tag="ob")
                so = iop.tile([8, ACH], BF16, tag="so")
                sub = 0
                for q0 in range(0, nn, 512):
                    q1 = min(q0 + 512, nn)
                    nq = q1 - q0
                    pxp = psa.tile([128, 512], F32, tag="xp")
                    nc.tensor.matmul(out=pxp[:, 0:nq], lhsT=rw_t[:],
                                     rhs=xt[:, q0:q1], start=True, stop=True)
                    ph = psb.tile([128, 512], F32, tag="h")
                    nc.tensor.matmul(out=ph[:, 0:nq], lhsT=rw1_t[:],
                                     rhs=xt[:, q0:q1], start=True, stop=True)
                    psd = psc.tile([8, 512], F32, tag="sd")
                    nc.tensor.matmul(out=psd[:, 0:nq], lhsT=ra_t[:],
                                     rhs=xt[:, q0:q1], start=True, stop=True)
                    nc.scalar.activation(out=ob[:, 0, q0:q1], in_=pxp[:, 0:nq],
                                         func=AF.Identity, bias=rb_t[:])
                    nc.vector.tensor_copy(ob[:, 1, q0:q1], ph[:, 0:nq])
                    if sub % 2 == 0:
                        nc.scalar.activation(out=so[:, q0:q1],
                                             in_=psd[:, 0:nq],
                                             func=AF.Identity, bias=ba_t[:])
                    else:
                        nc.vector.tensor_scalar(out=so[:, q0:q1],
                                                in0=psd[:, 0:nq],
                                                scalar1=ba_t[:], scalar2=None,
                                                op0=ALU.add)
                    sub += 1
                nc.sync.dma_start(out=o1[:, :, c0:c1], in_=ob[:, :, 0:nn])
                nc.gpsimd.dma_start(out=sdT[:, c0:c1], in_=so[:, 0:nn])
    nc.compile()
    return nc


def _edge_inputs(nc, prep):
    aps = {}
    aps["P"] = nc.dram_tensor("P", [128, prep.TT, D], BF16,
                              kind="ExternalInput").ap()
    aps["spack"] = nc.dram_tensor("spack", [128, prep.TT * NH], BF16,
                                  kind="ExternalInput").ap()
    aps["dtab"] = nc.dram_tensor("dtab", [128, S, NH], BF16,
                                 kind="ExternalInput").ap()
    aps["ident"] = nc.dram_tensor("ident", [128, 128], BF16,
                                  kind="ExternalInput").ap()
    aps["crep3"] = nc.dram_tensor("crep3", [128, 3, 128], BF16,
                                  kind="ExternalInput").ap()
    return aps


def _edge_phase(nc, prep, aps, tail_fn, pools):
    """Software-pipelined edge phase (head AHEAD groups early)."""
    cp, iop, wp, up = pools["cp"], pools["iop"], pools["wp"], pools["up"]
    psagg = pools["psagg"]
    ident = cp.tile([128, 128], BF16)
    nc.sync.dma_start(out=ident[:], in_=aps["ident"])
    crep3 = cp.tile([128, 3, 128], BF16)
    nc.sync.dma_start(out=crep3[:], in_=aps["crep3"])
    dtab = cp.tile([128, S, NH], BF16)
    nc.sync.dma_start(out=dtab[:], in_=aps["dtab"])
    spk = cp.tile([128, prep.TT * NH], BF16)
    nc.sync.dma_start(out=spk[:], in_=aps["spack"])
    pools["ident"] = ident

    AF = mybir.ActivationFunctionType
    ALU = mybir.AluOpType
    head_pk = {}
    head_pre = pools.get("head_pre")

    def head(g):
        s0, s1, t0, tg = prep.groups[g]
        nb = s1 - s0
        nt = nb * tg
        pk = iop.tile([128, TCAP, D], BF16, tag="pk")
        nc.sync.dma_start(out=pk[:, 0:nt, :], in_=aps["P"][:, t0:t0 + nt, :])
        if head_pre is not None:
            head_pre(g, s0, nb)

        # alpha chain from the resident score sidecar, (b, h, t) layout
        sc = spk[:, t0 * NH:(t0 + nt) * NH].rearrange(
            "p (b h t) -> p b h t", h=NH, t=tg)
        lg = wp.tile([128, TCAP * NH], BF16, tag="lg")
        lg_v = lg[:, 0:nt * NH].rearrange("p (b h t) -> p b h t", h=NH, t=tg)
        d_b = dtab[:, s0:s1, :].unsqueeze(3).to_broadcast([128, nb, NH, tg])
        nc.vector.tensor_tensor(out=lg_v, in0=sc, in1=d_b, op=ALU.add)
        ll = wp.tile([128, TCAP * NH], BF16, tag="ll")
        nc.scalar.activation(out=ll[:, 0:nt * NH], in_=lg[:, 0:nt * NH],
                             func=AF.Prelu, alpha=0.2)
        ex = wp.tile([128, TCAP * NH], BF16, tag="ex")
        ex_v = ex[:, 0:nt * NH].rearrange("p (b h t) -> p b h t", h=NH, t=tg)
        nc.scalar.activation(out=ex_v,
                             in_=ll[:, 0:nt * NH].rearrange(
                                 "p (b h t) -> p b h t", h=NH, t=tg),
                             func=AF.Exp)
        z = wp.tile([128, NBCAP * NH], F32, tag="z")
        z_v = z[:, 0:nb * NH].rearrange("p (b h) -> p b h", h=NH)
        nc.vector.tensor_reduce(out=z_v, in_=ex_v,
                                axis=mybir.AxisListType.X, op=ALU.add)
        zr = wp.tile([128, NBCAP * NH], BF16, tag="zr")
        zr_v = zr[:, 0:nb * NH].rearrange("p (b h) -> p b h", h=NH)
        with nc.allow_low_precision(reason="softmax z recip; 2e-2 budget"):
            nc.vector.reciprocal(zr_v, z_v)
        # normalized alpha into slot-major layout (strided write on the
        # otherwise-idle Pool engine)
        at = wp.tile([128, TCAP, NH], BF16, tag="at")
        at_bht = at[:, 0:nt, :].rearrange("p (b t) h -> p b h t", t=tg)
        zr_b = zr_v.unsqueeze(3).to_broadcast([128, nb, NH, tg])
        nc.gpsimd.tensor_tensor(out=at_bht, in0=ex_v, in1=zr_b, op=ALU.mult)

        # alpha-weight the value rows ((c,h) order, 2x DVE mode)
        vh = pk[:, 0:nt, :].rearrange("p t (c h) -> p t c h", h=NH)
        ab = at[:, 0:nt, :].unsqueeze(2).to_broadcast([128, nt, HD, NH])
        nc.vector.tensor_tensor(out=vh, in0=vh, in1=ab, op=ALU.mult)
        head_pk[g] = pk

    def tailw(g):
        s0, s1, t0, tg = prep.groups[g]
        nb = s1 - s0
        nt = nb * tg
        pk = head_pk.pop(g)
        vb = pk[:, 0:nt, :].rearrange("p (b t) f -> p b t f", t=tg)
        u_sb = up.tile([128, NBCAP, 128], BF16, tag="u")
        eg = up.tile([128, NBCAP, 128], BF16, tag="eg")
        for q0 in range(0, nb, 3):
            q1 = min(q0 + 3, nb)
            nq = q1 - q0
            ps = psagg.tile([128, 3, 128], F32, tag="agg")
            for t in range(tg):
                nc.tensor.matmul(out=ps[:, 0:nq, :], lhsT=ident[:],
                                 rhs=vb[:, q0:q1, t, :],
                                 start=(t == 0), stop=False)
            nc.tensor.matmul(out=ps[:, 0:nq, :], lhsT=ident[:],
                             rhs=crep3[:, 0:nq, :], start=False, stop=True)
            # ELU pieces straight from PSUM, both on ACT back-to-back
            nc.scalar.activation(out=u_sb[:, q0:q1, :], in_=ps[:, 0:nq, :],
                                 func=AF.Relu)
            nc.scalar.activation(out=eg[:, q0:q1, :], in_=ps[:, 0:nq, :],
                                 func=AF.Exp)
        uf = u_sb[:, 0:nb, :]
        nc.vector.scalar_tensor_tensor(out=uf, in0=eg[:, 0:nb, :],
                                       scalar=-1.0, in1=uf,
                                       op0=ALU.add, op1=ALU.min)
        tail_fn(s0, nb, u_sb, pools)

    G = len(prep.groups)
    AHEAD = 3
    for g in range(min(AHEAD, G)):
        head(g)
    for g in range(G):
        if g + AHEAD < G:
            head(g + AHEAD)
        tailw(g)


def build_launch_b(prep):
    """Layer-1 edge phase + layer-2 projection (h2cat = u1 @ W2cat)."""
    nc = bacc.Bacc("TRN2", target_bir_lowering=False, debug=False,
                   num_devices=NCORES)
    aps = _edge_inputs(nc, prep)
    aps["w2"] = nc.dram_tensor("w2", [128, 136], BF16,
                               kind="ExternalInput").ap()
    h2T = nc.dram_tensor("h2T", [128, SLOTS], BF16,
                         kind="ExternalOutput").ap()
    sd2T = nc.dram_tensor("sd2T", [8, SLOTS], BF16,
                          kind="ExternalOutput").ap()

    with tile.TileContext(nc) as tc:
        with (
            tc.tile_pool(name="const", bufs=1) as cp,
            tc.tile_pool(name="io", bufs=4) as iop,
            tc.tile_pool(name="wk", bufs=3) as wp,
            tc.tile_pool(name="u", bufs=3) as up,
            tc.tile_pool(name="og", bufs=3) as ogp,
            tc.tile_pool(name="psagg", bufs=2, space="PSUM") as psagg,
            tc.tile_pool(name="pst", bufs=2, space="PSUM") as pst,
            tc.tile_pool(name="psh", bufs=2, space="PSUM") as psh,
            tc.tile_pool(name="pssd", bufs=2, space="PSUM") as pssd,
        ):
            pools = dict(cp=cp, iop=iop, wp=wp, up=up, psagg=psagg)
            w2_t = cp.tile([128, 136], BF16)
            nc.sync.dma_start(out=w2_t[:], in_=aps["w2"])

            def tail(s0, nb, u_sb, pools):
                ident = pools["ident"]
                o1g = ogp.tile([128, NBCAP, 128], BF16, tag="o1")
                for i in range(nb):
                    pt = pst.tile([128, 128], BF16, tag="t")
                    nc.tensor.transpose(out=pt[:], in_=u_sb[:, i, :],
                                        identity=ident[:])
                    if i % 2 == 0:
                        nc.scalar.copy(o1g[:, i, :], pt[:])
                    else:
                        nc.vector.tensor_copy(o1g[:, i, :], pt[:])
                h2c = ogp.tile([128, NBCAP, 128], BF16, tag="h2c")
                s2c = ogp.tile([8, NBCAP, 128], BF16, tag="s2c")
                for q0 in range(0, nb, 4):
                    q1 = min(q0 + 4, nb)
                    k = (q1 - q0) * 128
                    rhs = o1g[:, q0:q1, :].rearrange("p b f -> p (b f)")
                    ph = psh.tile([128, 512], F32, tag="h")
                    nc.tensor.matmul(out=ph[:, 0:k], lhsT=w2_t[:, 0:128],
                                     rhs=rhs, start=True, stop=True)
                    nc.scalar.copy(
                        h2c[:, q0:q1, :].rearrange("p b f -> p (b f)"),
                        ph[:, 0:k])
                    psd = pssd.tile([8, 512], F32, tag="sd")
                    nc.tensor.matmul(out=psd[:, 0:k], lhsT=w2_t[:, 128:136],
                                     rhs=rhs, start=True, stop=True)
                    if (q0 // 4) % 2 == 0:
                        nc.scalar.copy(
                            s2c[:, q0:q1, :].rearrange("p b f -> p (b f)"),
                            psd[:, 0:k])
                    else:
                        nc.vector.tensor_copy(
                            s2c[:, q0:q1, :].rearrange("p b f -> p (b f)"),
                            psd[:, 0:k])
                nc.gpsimd.dma_start(out=h2T[:, s0 * 128:(s0 + nb) * 128],
                                    in_=h2c[:, 0:nb, :])
                nc.gpsimd.dma_start(out=sd2T[:, s0 * 128:(s0 + nb) * 128],
                                    in_=s2c[:, 0:nb, :])

            _edge_phase(nc, prep, aps, tail, pools)
    nc.compile()
    return nc


def build_launch_c(prep):
    """Layer-2 edge phase + residual + MLP head + log_softmax."""
    nc = bacc.Bacc("TRN2", target_bir_lowering=False, debug=False,
                   num_devices=NCORES)
    aps = _edge_inputs(nc, prep)
    aps["wc1"] = nc.dram_tensor("wc1", [128, 64], BF16,
                                kind="ExternalInput").ap()
    aps["cccol"] = nc.dram_tensor("cccol", [64, 1], F32,
                                  kind="ExternalInput").ap()
    aps["wc2aug"] = nc.dram_tensor("wc2aug", [65, 40], BF16,
                                   kind="ExternalInput").ap()
    aps["xp"] = nc.dram_tensor("xp", [128, S, 128], BF16,
                               kind="ExternalInput").ap()
    fin = nc.dram_tensor("fin", [128, S, 40], BF16,
                         kind="ExternalOutput").ap()

    AF = mybir.ActivationFunctionType
    ALU = mybir.AluOpType
    with tile.TileContext(nc) as tc:
        with (
            tc.tile_pool(name="const", bufs=1) as cp,
            tc.tile_pool(name="io", bufs=4) as iop,
            tc.tile_pool(name="wk", bufs=3) as wp,
            tc.tile_pool(name="u", bufs=3) as up,
            tc.tile_pool(name="og", bufs=3) as ogp,
            tc.tile_pool(name="psagg", bufs=2, space="PSUM") as psagg,
            tc.tile_pool(name="pst", bufs=2, space="PSUM") as pst,
            tc.tile_pool(name="psr", bufs=2, space="PSUM") as psr,
            tc.tile_pool(name="psy", bufs=2, space="PSUM") as psy,
        ):
            pools = dict(cp=cp, iop=iop, wp=wp, up=up, psagg=psagg)
            wc1_t = cp.tile([128, 64], BF16)
            nc.sync.dma_start(out=wc1_t[:], in_=aps["wc1"])
            cc_t = cp.tile([64, 1], F32)
            nc.sync.dma_start(out=cc_t[:], in_=aps["cccol"])
            wc2_t = cp.tile([65, 40], BF16)
            nc.sync.dma_start(out=wc2_t[:], in_=aps["wc2aug"])

            xpts = {}

            def head_pre(g, s0, nb):
                xpt = iop.tile([128, NBCAP, 128], BF16, tag="xpt")
                xpts[g % 4] = xpt
                nc.sync.dma_start(out=xpt[:, 0:nb, :],
                                  in_=aps["xp"][:, s0:s0 + nb, :])
            pools["head_pre"] = head_pre
            gcount = [0]
            y_all = cp.tile([128, S, 40], BF16)
            zs_all = cp.tile([128, S], F32)

            def tail(s0, nb, u_sb, pools):
                ident = pools["ident"]
                xpt = xpts[gcount[0] % 4]
                gcount[0] += 1
                uf = u_sb[:, 0:nb, :]
                nc.vector.tensor_tensor(out=uf, in0=uf,
                                        in1=xpt[:, 0:nb, :], op=ALU.add)
                o2g = ogp.tile([128, NBCAP, 128], BF16, tag="o2")
                for i in range(nb):
                    pt = pst.tile([128, 128], BF16, tag="t")
                    nc.tensor.transpose(out=pt[:], in_=u_sb[:, i, :],
                                        identity=ident[:])
                    if i % 2 == 0:
                        nc.scalar.copy(o2g[:, i, :], pt[:])
                    else:
                        nc.vector.tensor_copy(o2g[:, i, :], pt[:])
                r1g = ogp.tile([65, NBCAP, 128], BF16, tag="r1")
                nc.gpsimd.memset(r1g[64:65, 0:nb, :], 1.0)
                for q0 in range(0, nb, 4):
                    q1 = min(q0 + 4, nb)
                    k = (q1 - q0) * 128
                    rhs = o2g[:, q0:q1, :].rearrange("p b f -> p (b f)")
                    prt = psr.tile([64, 512], F32, tag="r")
                    nc.tensor.matmul(out=prt[:, 0:k], lhsT=wc1_t[:],
                                     rhs=rhs, start=True, stop=True)
                    nc.scalar.activation(
                        out=r1g[0:64, q0:q1, :].rearrange("p b f -> p (b f)"),
                        in_=prt[:, 0:k], func=AF.Relu, bias=cc_t[:])
                pyg = psy.tile([128, NBCAP, 40], F32, tag="py")
                for i in range(nb):
                    nc.tensor.matmul(out=pyg[:, i, :], lhsT=r1g[:, i, :],
                                     rhs=wc2_t[:], start=True, stop=True)
                # softmax pieces only; Ln deferred to one end-of-launch
                # pass (per-group Ln thrashes the ACT function table)
                egy = wp.tile([128, NBCAP, 40], BF16, tag="egy")
                nc.scalar.activation(out=egy[:, 0:nb, :], in_=pyg[:, 0:nb, :],
                                     func=AF.Exp)
                nc.vector.tensor_reduce(out=zs_all[:, s0:s0 + nb]
                                        .unsqueeze(2), in_=egy[:, 0:nb, :],
                                        axis=mybir.AxisListType.X, op=ALU.add)
                nc.scalar.copy(y_all[:, s0:s0 + nb, :], pyg[:, 0:nb, :])

            _edge_phase(nc, prep, aps, tail, pools)
            lz_all = cp.tile([128, S], F32)
            nc.scalar.activation(out=lz_all[:].unsqueeze(2),
                                 in_=zs_all[:].unsqueeze(2), func=AF.Ln)
            fin_all = cp.tile([128, S, 40], BF16)
            lz_b = lz_all[:].unsqueeze(2).to_broadcast([128, S, 40])
            nc.vector.tensor_tensor(out=fin_all[:], in0=y_all[:], in1=lz_b,
                                    op=ALU.subtract)
            nc.gpsimd.dma_start(out=fin, in_=fin_all[:])
    nc.compile()
    return nc


# ----------------------------------------------------------------------------
# Host orchestration
# ----------------------------------------------------------------------------

_cache = {}


def _get(key, fn):
    if key not in _cache:
        _cache[key] = fn()
    return _cache[key]


def _amat(a):
    """[NH, HD] attention vector -> [128, NH] block matrix."""
    m = np.zeros((D, NH), np.float32)
    for h in range(NH):
        m[h * HD:(h + 1) * HD, h] = a[h]
    return m


def _run(nc, in_maps, tag):
    res = run_bass_kernel_spmd(nc, in_maps, list(range(NCORES)),
                               trace=PROFILE)
    if PROFILE:
        LAST_EXEC_NS.append((tag, res.exec_time_ns))
    return res.results


def _fold_bn(g_, be_, rm_, rv_, bias):
    k = (g_ / np.sqrt(rv_ + EPS_BN)).astype(np.float32)
    c = ((bias - rm_) * k + be_).astype(np.float32)
    return k, c


def kernel(x, edge_index, res_W, res_b,
           W1, as1, ad1, b1, g1, be1, rm1, rv1,
           W2, as2, ad2, b2, g2, be2, rm2, rv2,
           Wc1, bc1, gc, bec, rmc, rvc, Wc2, bc2):
    bf = _bf()
    x = np.asarray(x, np.float32)
    edge_index = np.asarray(edge_index)

    ekey = ("prep", hash(edge_index.tobytes()))
    prep = _get(ekey, lambda: host_prep(edge_index.astype(np.int64)))

    k1, c1 = _fold_bn(g1, be1, rm1, rv1, b1)
    k2, c2 = _fold_bn(g2, be2, rm2, rv2, b2)
    kc, cc = _fold_bn(gc, bec, rmc, rvc, bc1)

    res_W = np.asarray(res_W, np.float32)
    res_b = np.asarray(res_b, np.float32)
    W1k = np.asarray(W1, np.float32) * k1[None, :]
    A1cat = np.concatenate([W1 @ _amat(as1), W1 @ _amat(ad1)], axis=1)
    RW1 = (res_W @ W1k).astype(bf)
    RA = (res_W @ A1cat).astype(bf)
    ba = (res_b @ A1cat).astype(np.float32)
    rb1k = (res_b @ W1k).astype(np.float32)  # h1 bias, folded into crep1
    W2cat = np.concatenate(
        [np.asarray(W2, np.float32) * k2[None, :],
         W2 @ _amat(as2), W2 @ _amat(ad2)], axis=1)[PERM, :].astype(bf)
    Wc1p = (np.asarray(Wc1, np.float32) * kc[None, :])[PERM, :].astype(bf)
    Wc2aug = np.concatenate(
        [np.asarray(Wc2, np.float32),
         np.asarray(bc2, np.float32).reshape(1, OUT)]).astype(bf)
    ident = np.eye(128, dtype=bf)
    rep3 = lambda v: np.tile(np.asarray(v, np.float32).astype(bf),
                             (128, 3)).reshape(128, 3, 128)
    crep3_1 = rep3((c1 + rb1k)[PERM])
    crep3_2 = rep3(c2[PERM])

    # ---- launch A: node projections ----
    x_pad = np.concatenate([x, np.zeros((1, IN), np.float32)]).astype(bf)
    nc_a = _get("A", build_launch_a)
    in_a = []
    for c in range(NCORES):
        idx = np.where(prep.valid_c[c], prep.order[
            np.minimum(prep.ranks_c[c], N - 1)], N)
        xs = np.ascontiguousarray(x_pad[idx].T)
        in_a.append(dict(xT=xs, rw=res_W.astype(bf),
                         rbcol=res_b.reshape(128, 1),
                         rw1=RW1, ra=RA, bacol=ba.reshape(8, 1)))
    res_a = _run(nc_a, in_a, "A")

    # assemble rank-ordered h1 / s1 / d1 / xp
    h1_rank = np.zeros((N, 128), bf)
    sd1_rank = np.zeros((N, 8), np.float32)
    xp_rows = []
    for c in range(NCORES):
        v = prep.valid_c[c]
        rc = prep.ranks_c[c][v]
        o = np.asarray(res_a[c]["o1"])  # [128, 2, SLOTS] bf16
        h1_rank[rc] = o[:, 1, :].T[v]
        sd1_rank[rc] = np.asarray(res_a[c]["sdT"]).T[v].astype(np.float32)
        xr = np.ascontiguousarray(
            o[PERM, 0, :].T.reshape(S, 128, 128).transpose(1, 0, 2))
        xp_rows.append(xr)

    def table_of(h_rank_bf):
        t = np.zeros((N + 1, D), np.uint16)
        t[:N] = h_rank_bf[:, PERM].view(np.uint16)
        return t

    # ---- launch B: layer-1 edge phase + layer-2 projection ----
    P1 = build_pack(prep, table_of(h1_rank))
    s1p = build_spack(prep, sd1_rank[:, 0:4])
    d1 = build_dtab(prep, sd1_rank[:, 4:8])
    nc_b = _get(("B", prep.TT), lambda: build_launch_b(prep))
    in_b = [dict(P=P1[c].view(bf), spack=s1p[c], dtab=d1[c], ident=ident,
                 crep3=crep3_1, w2=W2cat) for c in range(NCORES)]
    res_b_ = _run(nc_b, in_b, "B")

    h2_rank = np.zeros((N, 128), bf)
    sd2_rank = np.zeros((N, 8), np.float32)
    for c in range(NCORES):
        v = prep.valid_c[c]
        rc = prep.ranks_c[c][v]
        h2_rank[rc] = np.asarray(res_b_[c]["h2T"]).T[v]
        sd2_rank[rc] = np.asarray(res_b_[c]["sd2T"]).T[v].astype(np.float32)

    # ---- launch C: layer-2 edge phase + residual + head ----
    P2 = build_pack(prep, table_of(h2_rank))
    s2p = build_spack(prep, sd2_rank[:, 0:4])
    d2 = build_dtab(prep, sd2_rank[:, 4:8])
    nc_c = _get(("C", prep.TT), lambda: build_launch_c(prep))
    in_c = [dict(P=P2[c].view(bf), spack=s2p[c], dtab=d2[c], ident=ident,
                 crep3=crep3_2,
                 wc1=Wc1p, cccol=cc.reshape(64, 1).astype(np.float32),
                 wc2aug=Wc2aug, xp=xp_rows[c])
            for c in range(NCORES)]
    res_c = _run(nc_c, in_c, "C")

    out_rank = np.zeros((N, OUT), np.float32)
    for c in range(NCORES):
        v = prep.valid_c[c]
        rc = prep.ranks_c[c][v]
        f = np.asarray(res_c[c]["fin"]).astype(np.float32)
        out_rank[rc] = f.transpose(1, 0, 2).reshape(SLOTS, OUT)[v]
    out = np.empty((N, OUT), np.float32)
    out[prep.order] = out_rank
    return out
